# revision 9
# baseline (speedup 1.0000x reference)
"""Multi-head causal attention (B=4, S=2048, D=1024, H=16) on 8 TRN2 NeuronCores.

Sharding: data-parallel over batch (4) x tensor-parallel over heads (2 groups
of 8 heads) = 8 cores. Each core computes, for its (batch, head-group):
  Q^T/K^T = (x @ Wq/Wk)^T   [dc, S]   (dc = 512 head-group dims)
  V       = x @ Wv          [S, dc]
  per head h, per 512-wide query block iB (flash-style, scores transposed):
    E[j, i]      = exp(scoresT / 8) with causal mask (j <= i), j tiled by 128
    attnoutT|r   = [V_h | ones].T @ E  -> [65, i]  (row 64 = softmax denom r)
    anorm        = attnoutT * (1/r)    (broadcast over d)
  out_partial[i, :] += anorm_h.T @ (W_o[:, cols].T)   accumulated over heads
Host sums the two head-group partials per batch (the W_o row-shard
all-reduce from the sharding hint, done on host during unshard).

Schedule: projection block sb and attention query-block iB=sb are
interleaved — causal attention for queries [512*sb, 512*(sb+1)) needs
exactly the K/V produced by projection blocks <= sb.  This keeps the PE
dense (projection matmuls fill the exp-bound gaps of early attention
blocks) so the HAM activity monitor never re-throttles the PE clock.

The last two key chunks of each diagonal block are computed only for the
query half that can attend to them (staircase), halving their score/exp/
attend cost.

Q/K/V/E and the QKV projections run in bf16 (halves DMA + SBUF traffic,
enables fast weight loads); PSUM accumulation is fp32; the output
projection runs in fp32r for precision at the last layer.
"""

import sys

if "/opt/trn_rl_repo" not in sys.path:
    sys.path.insert(0, "/opt/trn_rl_repo")

import ml_dtypes
import numpy as np

import concourse.bacc as bacc
import concourse.mybir as mybir
import concourse.tile as tile
from concourse.bass import ts
from concourse.bass_utils import run_bass_kernel_spmd

F32 = mybir.dt.float32
F32R = mybir.dt.float32r
BF16 = mybir.dt.bfloat16
AF = mybir.ActivationFunctionType
BFNP = np.dtype(ml_dtypes.bfloat16)

B, S, D, H = 4, 2048, 1024, 16
HD = D // H           # 64
NCORES = 8
HG = 8                # heads per core
DC = HG * HD          # 512 feature cols per core
SB = 512              # s-block
NSB = S // SB         # 4
KC = D // 128         # 8 k-chunks
NIB = S // 512        # 4 query blocks
SCALE = 1.0 / np.sqrt(HD)

_cached_nc = None


def _build():
    nc = bacc.Bacc("TRN2", target_bir_lowering=False, debug=False)

    xt_d = nc.dram_tensor("xt", [D, S], BF16, kind="ExternalInput")      # x[b].T
    wq_d = nc.dram_tensor("wq", [D, DC], BF16, kind="ExternalInput")
    wk_d = nc.dram_tensor("wk", [D, DC], BF16, kind="ExternalInput")
    wv_d = nc.dram_tensor("wv", [D, DC], BF16, kind="ExternalInput")
    wot_d = nc.dram_tensor("wot", [DC, D], F32R, kind="ExternalInput")   # W_o[:, cols].T
    out_d = nc.dram_tensor("out", [S, D], F32, kind="ExternalOutput")

    with tile.TileContext(nc) as tc:
        with (
            tc.tile_pool(name="qtkt", bufs=4) as qtkt_pool,
            tc.tile_pool(name="vp", bufs=16) as v_pool,
            tc.tile_pool(name="wp", bufs=1) as w_pool,
            tc.tile_pool(name="xtp", bufs=2) as xt_pool,
            tc.tile_pool(name="wotp", bufs=8) as wot_pool,
            tc.tile_pool(name="ep", bufs=4) as e_pool,
            tc.tile_pool(name="recp", bufs=4) as rec_pool,
            tc.tile_pool(name="anp", bufs=6) as an_pool,
            tc.tile_pool(name="bcp", bufs=4) as bc_pool,
            tc.tile_pool(name="op", bufs=2) as o_pool,
            tc.tile_pool(name="ps_mm", bufs=2, space="PSUM") as ps_mm,
            tc.tile_pool(name="ps_acc", bufs=2, space="PSUM") as ps_acc,
            tc.tile_pool(name="ps_proj", bufs=2, space="PSUM") as ps_proj,
        ):
            # persistent tiles
            qt = [qtkt_pool.tile([128, S], BF16, tag="qt", name=f"qt{i}") for i in range(4)]
            kt = [qtkt_pool.tile([128, S], BF16, tag="kt", name=f"kt{i}") for i in range(4)]
            vt = [v_pool.tile([128, HG, HD + 1], BF16, tag="vt", name=f"vt{i}") for i in range(16)]
            ones8 = v_pool.tile([128, HG], BF16, tag="ones8", bufs=1)
            nc.vector.memset(ones8, 1.0)

            wot = [wot_pool.tile([128, D], F32R, tag="wot", name=f"wot{i}") for i in range(4)]

            wq = w_pool.tile([128, KC, DC], BF16, tag="wq")
            wk = w_pool.tile([128, KC, DC], BF16, tag="wk")
            wv = w_pool.tile([128, KC, DC], BF16, tag="wv")
            # first projection block's x lands first, interleaved per k-chunk
            # with the weights it multiplies, so the PE starts within ~2us
            xt0 = xt_pool.tile([128, KC, SB], BF16, tag="xt", name="xt_sb0")
            for kc in range(KC):
                nc.sync.dma_start(out=xt0[:, kc, :], in_=xt_d[ts(kc, 128), ts(0, SB)])
                nc.sync.dma_start(out=wq[:, kc, :], in_=wq_d[ts(kc, 128), :])
                nc.sync.dma_start(out=wk[:, kc, :], in_=wk_d[ts(kc, 128), :])
                nc.sync.dma_start(out=wv[:, kc, :], in_=wv_d[ts(kc, 128), :])
            for t in range(4):
                nc.sync.dma_start(out=wot[t], in_=wot_d[ts(t, 128), :])

            for sb in range(NSB):
                # ---------- projection block sb ----------
                if sb == 0:
                    xt_t = xt0
                else:
                    xt_t = xt_pool.tile([128, KC, SB], BF16, tag="xt",
                                        name=f"xt_sb{sb}")
                    for kc in range(KC):
                        nc.sync.dma_start(
                            out=xt_t[:, kc, :],
                            in_=xt_d[ts(kc, 128), ts(sb, SB)],
                        )
                # Q^T, K^T: [dc-chunk(128), s-block] accumulated over k
                for w_t, dst in ((wq, qt), (wk, kt)):
                    for m in range(4):
                        ps = ps_mm.tile([128, SB], F32, tag="mm")
                        for kc in range(KC):
                            nc.tensor.matmul(
                                ps,
                                w_t[:, kc, ts(m, 128)],
                                xt_t[:, kc, :],
                                start=(kc == 0), stop=(kc == KC - 1),
                            )
                        nc.vector.tensor_copy(dst[m][:, ts(sb, SB)], ps)
                # V natural [s-chunk(128), dc] accumulated over k
                for sc in range(4):
                    ps = ps_mm.tile([128, DC], F32, tag="mm")
                    for kc in range(KC):
                        nc.tensor.matmul(
                            ps,
                            xt_t[:, kc, ts(sc, 128)],
                            wv[:, kc, :],
                            start=(kc == 0), stop=(kc == KC - 1),
                        )
                    v_t = vt[4 * sb + sc]
                    nc.vector.tensor_copy(
                        v_t[:, :, 0:HD],
                        ps.rearrange("p (h d) -> p h d", h=HG),
                    )
                    nc.vector.tensor_copy(v_t[:, :, HD:HD + 1], ones8)

                # ---------- attention query block iB = sb ----------
                iB = sb
                nu = 2 * iB + 2        # jj pairs incl. the diagonal pair
                anorms = [None] * 4
                for hp in range(HG // 2):
                    h0, h1 = 2 * hp, 2 * hp + 1
                    m = hp
                    accs = {h0: ps_acc.tile([128, SB], F32, tag="acc", name=f"acc0_{iB}_{hp}"),
                            h1: ps_acc.tile([128, SB], F32, tag="acc", name=f"acc1_{iB}_{hp}")}
                    for u in range(nu):
                        jj0, jj1 = 2 * u, 2 * u + 1
                        # staircase: the last chunk pair (keys in
                        # [512iB+256, 512iB+512)) can only be attended by the
                        # top query half — compute just those 256 queries
                        half = (u == nu - 1)
                        qw = SB // 2 if half else SB     # queries per chunk
                        qoff = iB * SB + (SB // 2 if half else 0)
                        pss = {h: ps_mm.tile([128, 1024], F32, tag="mm",
                                             name=f"ps_{iB}_{hp}_{u}_{h}")
                               for h in (h0, h1)}
                        # alternate heads so consecutive matmuls hit disjoint
                        # PE row groups (h0: rows 0-63, h1: rows 64-127) and
                        # run concurrently in the array
                        for q, jj in ((0, jj0), (1, jj1)):
                            for h in (h0, h1):
                                rb = (h % 2) * 64
                                nc.tensor.matmul(
                                    pss[h][:, q * qw:(q + 1) * qw],
                                    kt[m][rb:rb + 64, ts(jj, 128)],
                                    qt[m][rb:rb + 64, qoff:qoff + qw],
                                    start=True, stop=True,
                                )
                        for h in (h0, h1):
                            e_t = e_pool.tile([128, 1024], BF16, tag="e",
                                              name=f"e_{iB}_{hp}_{u}_{h}")
                            nc.scalar.activation(e_t[:, 0:2 * qw], pss[h][:, 0:2 * qw],
                                                 AF.Exp, scale=float(SCALE))
                            t0 = jj0 - 4 * iB
                            if t0 >= 0:
                                # keep key (128*(t0+c) + p) <= query q
                                nc.gpsimd.affine_select(
                                    out=e_t[:, 0:2 * qw], in_=e_t[:, 0:2 * qw],
                                    pattern=[[-128, 2], [1, qw]],
                                    compare_op=mybir.AluOpType.is_ge,
                                    fill=0.0,
                                    base=-128 * t0 + (qoff - iB * SB),
                                    channel_multiplier=-1,
                                )
                            for q, jj in ((0, jj0), (1, jj1)):
                                nc.tensor.matmul(
                                    accs[h][0:HD + 1, qoff - iB * SB:
                                            qoff - iB * SB + qw],
                                    vt[jj][:, h, :],
                                    e_t[:, q * qw:(q + 1) * qw],
                                    start=(u == 0 and q == 0),
                                    stop=(u == nu - 1 and q == 1),
                                )
                    an_pair = an_pool.tile([128, SB], F32R, tag="an",
                                           name=f"an_{iB}_{hp}")
                    anorms[hp] = an_pair
                    for h in (h1, h0):
                        t65 = rec_pool.tile([HD + 1, SB], F32, tag="t65",
                                            name=f"t65_{iB}_{h}")
                        nc.vector.tensor_copy(t65, accs[h][0:HD + 1, :])
                        r_t = rec_pool.tile([1, SB], F32, tag="r",
                                            name=f"r_{iB}_{h}")
                        nc.sync.dma_start(out=r_t, in_=t65[HD:HD + 1, :])
                        rec_t = rec_pool.tile([1, SB], F32, tag="rec",
                                              name=f"rec_{iB}_{h}")
                        nc.vector.reciprocal_approx_fast(out=rec_t, in_=r_t)
                        bc_t = bc_pool.tile([HD, SB], F32, tag="bc",
                                            name=f"bc_{iB}_{h}")
                        nc.gpsimd.partition_broadcast(bc_t, rec_t)
                        if h == h0:
                            nc.vector.tensor_mul(an_pair[0:HD, :],
                                                 t65[0:HD, :], bc_t)
                        else:
                            an_odd = an_pool.tile([HD, SB], F32R, tag="anodd",
                                                  name=f"anodd_{iB}_{h}")
                            nc.vector.tensor_mul(an_odd, t65[0:HD, :], bc_t)
                            # stage odd head to partitions 64..127 (DMA can
                            # move across partitions; DVE cannot)
                            nc.sync.dma_start(out=an_pair[HD:128, :], in_=an_odd)

                # output projection for this query block, summed over heads
                for ic in range(4):
                    o_t = o_pool.tile([128, D], F32, tag="o")
                    for dh in range(2):
                        po = ps_proj.tile([128, 512], F32, tag="po")
                        for hp2 in range(4):
                            nc.tensor.matmul(
                                po,
                                anorms[hp2][:, ts(ic, 128)],
                                wot[hp2][:, ts(dh, 512)],
                                start=(hp2 == 0), stop=(hp2 == 3),
                            )
                        nc.vector.tensor_copy(o_t[:, ts(dh, 512)], po)
                    nc.sync.dma_start(
                        out=out_d[iB * SB + ic * 128:iB * SB + (ic + 1) * 128, :],
                        in_=o_t,
                    )

    nc.compile()
    return nc


def make_in_maps(x, W_q, W_k, W_v, W_o):
    x = np.asarray(x, dtype=np.float32)
    W_q = np.asarray(W_q, dtype=np.float32)
    W_k = np.asarray(W_k, dtype=np.float32)
    W_v = np.asarray(W_v, dtype=np.float32)
    W_o = np.asarray(W_o, dtype=np.float32)

    in_maps = []
    for c in range(NCORES):
        b, g = c // 2, c % 2
        cols = slice(g * DC, (g + 1) * DC)
        in_maps.append({
            "xt": np.ascontiguousarray(x[b].T).astype(BFNP),
            "wq": np.ascontiguousarray(W_q[:, cols]).astype(BFNP),
            "wk": np.ascontiguousarray(W_k[:, cols]).astype(BFNP),
            "wv": np.ascontiguousarray(W_v[:, cols]).astype(BFNP),
            "wot": np.ascontiguousarray(W_o[:, cols].T),
        })
    return in_maps


def kernel(x, W_q, W_k, W_v, W_o):
    global _cached_nc
    if _cached_nc is None:
        _cached_nc = _build()
    nc = _cached_nc

    in_maps = make_in_maps(x, W_q, W_k, W_v, W_o)
    res = run_bass_kernel_spmd(nc, in_maps, list(range(NCORES))).results
    out = np.empty((B, S, D), np.float32)
    for b in range(B):
        out[b] = res[2 * b]["out"] + res[2 * b + 1]["out"]
    return out


# revision 12
# speedup vs baseline: 1.1746x; 1.1746x over previous
"""Multi-head causal attention (B=4, S=2048, D=1024, H=16) on 8 TRN2 NeuronCores.

Sharding: data-parallel over batch (4) x tensor-parallel over heads (2 groups
of 8 heads) = 8 cores. Each core computes, for its (batch, head-group):
  Q^T/K^T = (x @ Wq/Wk)^T   [dc, S]   (dc = 512 head-group dims)
  V       = x @ Wv          [S, dc]
  per head h, per 512-wide query block iB (flash-style, scores transposed):
    E[j, i]      = exp(scoresT / 8) with causal mask (j <= i), j tiled by 128
    attnoutT|r   = [V_h | ones].T @ E  -> [65, i]  (row 64 = softmax denom r)
    anorm        = attnoutT * (1/r)    (broadcast over d)
  out_partial[i, :] += anorm_h.T @ (W_o[:, cols].T)   accumulated over heads
Host sums the two head-group partials per batch (the W_o row-shard
all-reduce from the sharding hint, done on host during unshard).

Schedule: engines execute their instruction queues in order, so emission
order IS the schedule.  The attention stream (scores -> exp on the scalar
engine -> attend) is exp-bound: per u-step the scalar engine needs ~2.2us
while the attention matmuls only need ~1us.  To keep the PE dense (and the
HAM activity monitor from re-throttling its clock), every u-step pumps one
independent "filler" PE chunk — a QKV-projection chunk for a later key
block, or a deferred output-projection group from an earlier query block —
from a deadline-ordered queue.  Causality guarantees projection block k is
only needed by attention query-block k, so projection work for block k+1
fills the exp-bound gaps of attention block k.

The last two key chunks of each diagonal block are computed only for the
query half that can attend to them (staircase).  Score matmuls alternate
heads so consecutive matmuls hit disjoint PE row groups (K=64 contraction)
and run concurrently in the array.

Q/K/V/E and the QKV projections run in bf16 (halves DMA + SBUF traffic,
enables fast weight loads); PSUM accumulation is fp32; the output
projection runs in fp32r for precision at the last layer.
"""

import sys

if "/opt/trn_rl_repo" not in sys.path:
    sys.path.insert(0, "/opt/trn_rl_repo")

from collections import deque

import ml_dtypes
import numpy as np

import concourse.bacc as bacc
import concourse.mybir as mybir
import concourse.tile as tile
from concourse.bass import ts
from concourse.bass_utils import run_bass_kernel_spmd

F32 = mybir.dt.float32
F32R = mybir.dt.float32r
BF16 = mybir.dt.bfloat16
AF = mybir.ActivationFunctionType
BFNP = np.dtype(ml_dtypes.bfloat16)

B, S, D, H = 4, 2048, 1024, 16
HD = D // H           # 64
NCORES = 8
HG = 8                # heads per core
DC = HG * HD          # 512 feature cols per core
SB = 512              # s-block
NSB = S // SB         # 4
KC = D // 128         # 8 k-chunks
NIB = S // 512        # 4 query blocks
SCALE = 1.0 / np.sqrt(HD)
END = (99, 0, 0)      # deadline for discretionary fillers

_cached_nc = None


def _build():
    nc = bacc.Bacc("TRN2", target_bir_lowering=False, debug=False)

    xt_d = nc.dram_tensor("xt", [D, S], BF16, kind="ExternalInput")      # x[b].T
    wq_d = nc.dram_tensor("wq", [D, DC], BF16, kind="ExternalInput")
    wk_d = nc.dram_tensor("wk", [D, DC], BF16, kind="ExternalInput")
    wv_d = nc.dram_tensor("wv", [D, DC], BF16, kind="ExternalInput")
    wot_d = nc.dram_tensor("wot", [DC, D], F32R, kind="ExternalInput")   # W_o[:, cols].T
    out_d = nc.dram_tensor("out", [S, D], F32, kind="ExternalOutput")

    with tile.TileContext(nc) as tc:
        with (
            tc.tile_pool(name="qtkt", bufs=4) as qtkt_pool,
            tc.tile_pool(name="vp", bufs=16) as v_pool,
            tc.tile_pool(name="wp", bufs=1) as w_pool,
            tc.tile_pool(name="xtp", bufs=2) as xt_pool,
            tc.tile_pool(name="wotp", bufs=8) as wot_pool,
            tc.tile_pool(name="ep", bufs=4) as e_pool,
            tc.tile_pool(name="recp", bufs=4) as rec_pool,
            tc.tile_pool(name="anp", bufs=10) as an_pool,
            tc.tile_pool(name="bcp", bufs=4) as bc_pool,
            tc.tile_pool(name="op", bufs=2) as o_pool,
            tc.tile_pool(name="ps_mm", bufs=2, space="PSUM") as ps_mm,
            tc.tile_pool(name="ps_fill", bufs=2, space="PSUM") as ps_fill,
            tc.tile_pool(name="ps_acc", bufs=2, space="PSUM") as ps_acc,
        ):
            # persistent tiles
            qt = [qtkt_pool.tile([128, S], BF16, tag="qt", name=f"qt{i}") for i in range(4)]
            kt = [qtkt_pool.tile([128, S], BF16, tag="kt", name=f"kt{i}") for i in range(4)]
            vt = [v_pool.tile([128, HG, HD + 1], BF16, tag="vt", name=f"vt{i}") for i in range(16)]
            ones8 = v_pool.tile([128, HG], BF16, tag="ones8", bufs=1)
            nc.vector.memset(ones8, 1.0)
            wot = [wot_pool.tile([128, D], F32R, tag="wot", name=f"wot{i}") for i in range(4)]

            wq = w_pool.tile([128, KC, DC], BF16, tag="wq")
            wk = w_pool.tile([128, KC, DC], BF16, tag="wk")
            wv = w_pool.tile([128, KC, DC], BF16, tag="wv")
            # first block's x lands first, interleaved per k-chunk with the
            # weights it multiplies, so the PE starts within ~2us
            xts = [None] * NSB
            xts[0] = xt_pool.tile([128, KC, SB], BF16, tag="xt", name="xt_sb0")
            for kc in range(KC):
                nc.sync.dma_start(out=xts[0][:, kc, :], in_=xt_d[ts(kc, 128), ts(0, SB)])
                nc.sync.dma_start(out=wq[:, kc, :], in_=wq_d[ts(kc, 128), :])
                nc.sync.dma_start(out=wk[:, kc, :], in_=wk_d[ts(kc, 128), :])
                nc.sync.dma_start(out=wv[:, kc, :], in_=wv_d[ts(kc, 128), :])
            for t in range(4):
                nc.sync.dma_start(out=wot[t], in_=wot_d[ts(t, 128), :])

            # ---------------- filler chunk emitters ----------------
            def emit_qtkt(sb, m, w_t, dst, which):
                ps = ps_fill.tile([128, SB], F32, tag="fill",
                                  name=f"pj_{which}{sb}m{m}")
                for kc in range(KC):
                    nc.tensor.matmul(
                        ps, w_t[:, kc, ts(m, 128)], xts[sb][:, kc, :],
                        start=(kc == 0), stop=(kc == KC - 1),
                    )
                nc.vector.tensor_copy(dst[m][:, ts(sb, SB)], ps)

            def emit_v(sb, sc):
                ps = ps_fill.tile([128, DC], F32, tag="fill", name=f"pj_v{sb}s{sc}")
                for kc in range(KC):
                    nc.tensor.matmul(
                        ps, xts[sb][:, kc, ts(sc, 128)], wv[:, kc, :],
                        start=(kc == 0), stop=(kc == KC - 1),
                    )
                v_t = vt[4 * sb + sc]
                nc.vector.tensor_copy(
                    v_t[:, :, 0:HD], ps.rearrange("p (h d) -> p h d", h=HG))
                nc.vector.tensor_copy(v_t[:, :, HD:HD + 1], ones8)

            o_tiles = {}

            def emit_oproj(iB, anorms, ic, dh):
                if dh == 0:
                    o_tiles[(iB, ic)] = o_pool.tile([128, D], F32, tag="o",
                                                    name=f"o_{iB}_{ic}")
                o_t = o_tiles[(iB, ic)]
                po = ps_fill.tile([128, 512], F32, tag="fill", name=f"po_{iB}_{ic}_{dh}")
                for hp2 in range(4):
                    nc.tensor.matmul(
                        po, anorms[hp2][:, ts(ic, 128)], wot[hp2][:, ts(dh, 512)],
                        start=(hp2 == 0), stop=(hp2 == 3),
                    )
                nc.vector.tensor_copy(o_t[:, ts(dh, 512)], po)
                if dh == 1:
                    nc.sync.dma_start(
                        out=out_d[iB * SB + ic * 128:iB * SB + (ic + 1) * 128, :],
                        in_=o_t,
                    )

            # ---------------- deadline-ordered filler queue ----------------
            fillers = deque()       # (deadline_key, fn), deadlines non-decreasing

            def push_proj(sb):
                # x for block sb starts loading now; chunk order matches
                # first-use order inside attention block sb
                xts[sb] = xt_pool.tile([128, KC, SB], BF16, tag="xt",
                                       name=f"xt_sb{sb}")
                for kc in range(KC):
                    nc.sync.dma_start(out=xts[sb][:, kc, :],
                                      in_=xt_d[ts(kc, 128), ts(sb, SB)])
                fillers.append(((sb, 0, 0), lambda: emit_qtkt(sb, 0, wq, qt, "q")))
                fillers.append(((sb, 0, 0), lambda: emit_qtkt(sb, 0, wk, kt, "k")))
                for sc in range(4):
                    fillers.append(((sb, 0, 2 * sb + sc // 2),
                                    lambda sc=sc: emit_v(sb, sc)))
                for m in range(1, 4):
                    fillers.append(((sb, m, 0),
                                    lambda m=m: emit_qtkt(sb, m, wq, qt, "q")))
                    fillers.append(((sb, m, 0),
                                    lambda m=m: emit_qtkt(sb, m, wk, kt, "k")))

            def push_oproj(iB, anorms):
                for ic in range(4):
                    for dh in range(2):
                        fillers.append((END, lambda ic=ic, dh=dh:
                                        emit_oproj(iB, anorms, ic, dh)))

            def pump_due(key):
                while fillers and fillers[0][0] <= key:
                    fillers.popleft()[1]()

            def pump_one():
                if fillers:
                    fillers.popleft()[1]()

            # prelude: just enough of projection block 0 for the first item
            emit_qtkt(0, 0, wq, qt, "q")
            emit_qtkt(0, 0, wk, kt, "k")
            emit_v(0, 0)
            emit_v(0, 1)
            fillers.append(((0, 0, 1), lambda: emit_v(0, 2)))
            fillers.append(((0, 0, 1), lambda: emit_v(0, 3)))
            for m in range(1, 4):
                fillers.append(((0, m, 0), lambda m=m: emit_qtkt(0, m, wq, qt, "q")))
                fillers.append(((0, m, 0), lambda m=m: emit_qtkt(0, m, wk, kt, "k")))

            # ---------------- attention item stream ----------------
            for iB in range(NIB):
                if iB + 1 < NSB:
                    push_proj(iB + 1)
                if iB >= 1:
                    push_oproj(iB - 1, prev_anorms)

                nu = 2 * iB + 2        # jj pairs incl. the diagonal pair
                anorms = [None] * 4
                for hp in range(HG // 2):
                    h0, h1 = 2 * hp, 2 * hp + 1
                    m = hp
                    accs = {h0: ps_acc.tile([128, SB], F32, tag="acc", name=f"acc0_{iB}_{hp}"),
                            h1: ps_acc.tile([128, SB], F32, tag="acc", name=f"acc1_{iB}_{hp}")}
                    for u in range(nu):
                        pump_due((iB, hp, u))
                        jj0, jj1 = 2 * u, 2 * u + 1
                        # staircase: the last chunk pair (keys in
                        # [512iB+256, 512iB+512)) is only attended by the top
                        # query half — compute just those 256 queries
                        half = (u == nu - 1)
                        qw = SB // 2 if half else SB
                        qoff = iB * SB + (SB // 2 if half else 0)
                        pss = {h: ps_mm.tile([128, 1024], F32, tag="mm",
                                             name=f"ps_{iB}_{hp}_{u}_{h}")
                               for h in (h0, h1)}
                        # alternate heads: consecutive matmuls hit disjoint PE
                        # row groups (h0: rows 0-63, h1: 64-127) -> concurrent
                        for q, jj in ((0, jj0), (1, jj1)):
                            for h in (h0, h1):
                                rb = (h % 2) * 64
                                nc.tensor.matmul(
                                    pss[h][:, q * qw:(q + 1) * qw],
                                    kt[m][rb:rb + 64, ts(jj, 128)],
                                    qt[m][rb:rb + 64, qoff:qoff + qw],
                                    start=True, stop=True,
                                )
                        e_ts = {}
                        for h in (h0, h1):
                            e_t = e_pool.tile([128, 1024], BF16, tag="e",
                                              name=f"e_{iB}_{hp}_{u}_{h}")
                            e_ts[h] = e_t
                            nc.scalar.activation(e_t[:, 0:2 * qw], pss[h][:, 0:2 * qw],
                                                 AF.Exp, scale=float(SCALE))
                            t0 = jj0 - 4 * iB
                            if t0 >= 0:
                                # keep key (128*(t0+c) + p) <= query q
                                nc.gpsimd.affine_select(
                                    out=e_t[:, 0:2 * qw], in_=e_t[:, 0:2 * qw],
                                    pattern=[[-128, 2], [1, qw]],
                                    compare_op=mybir.AluOpType.is_ge,
                                    fill=0.0,
                                    base=-128 * t0 + (qoff - iB * SB),
                                    channel_multiplier=-1,
                                )
                        # independent PE work rides out the exp latency
                        pump_one()
                        for h in (h0, h1):
                            for q, jj in ((0, jj0), (1, jj1)):
                                nc.tensor.matmul(
                                    accs[h][0:HD + 1, qoff - iB * SB:
                                            qoff - iB * SB + qw],
                                    vt[jj][:, h, :],
                                    e_ts[h][:, q * qw:(q + 1) * qw],
                                    start=(u == 0 and q == 0),
                                    stop=(u == nu - 1 and q == 1),
                                )
                    an_pair = an_pool.tile([128, SB], F32R, tag="an",
                                           name=f"an_{iB}_{hp}")
                    anorms[hp] = an_pair
                    for h in (h1, h0):   # odd first: its DMA overlaps h0's chain
                        t65 = rec_pool.tile([HD + 1, SB], F32, tag="t65",
                                            name=f"t65_{iB}_{h}")
                        nc.vector.tensor_copy(t65, accs[h][0:HD + 1, :])
                        r_t = rec_pool.tile([1, SB], F32, tag="r",
                                            name=f"r_{iB}_{h}")
                        nc.sync.dma_start(out=r_t, in_=t65[HD:HD + 1, :])
                        rec_t = rec_pool.tile([1, SB], F32, tag="rec",
                                              name=f"rec_{iB}_{h}")
                        nc.vector.reciprocal_approx_fast(out=rec_t, in_=r_t)
                        bc_t = bc_pool.tile([HD, SB], F32, tag="bc",
                                            name=f"bc_{iB}_{h}")
                        nc.gpsimd.partition_broadcast(bc_t, rec_t)
                        if h == h0:
                            nc.vector.tensor_mul(an_pair[0:HD, :],
                                                 t65[0:HD, :], bc_t)
                        else:
                            an_odd = an_pool.tile([HD, SB], F32R, tag="anodd", bufs=4,
                                                  name=f"anodd_{iB}_{h}")
                            nc.vector.tensor_mul(an_odd, t65[0:HD, :], bc_t)
                            # stage odd head to partitions 64..127 (DMA can
                            # move across partitions; DVE cannot)
                            nc.sync.dma_start(out=an_pair[HD:128, :], in_=an_odd)
                prev_anorms = anorms

            # drain remaining fillers, then the last block's output projection
            while fillers:
                fillers.popleft()[1]()
            push_oproj(NIB - 1, prev_anorms)
            while fillers:
                fillers.popleft()[1]()

    nc.compile()
    return nc


def make_in_maps(x, W_q, W_k, W_v, W_o):
    x = np.asarray(x, dtype=np.float32)
    W_q = np.asarray(W_q, dtype=np.float32)
    W_k = np.asarray(W_k, dtype=np.float32)
    W_v = np.asarray(W_v, dtype=np.float32)
    W_o = np.asarray(W_o, dtype=np.float32)

    in_maps = []
    for c in range(NCORES):
        b, g = c // 2, c % 2
        cols = slice(g * DC, (g + 1) * DC)
        in_maps.append({
            "xt": np.ascontiguousarray(x[b].T).astype(BFNP),
            "wq": np.ascontiguousarray(W_q[:, cols]).astype(BFNP),
            "wk": np.ascontiguousarray(W_k[:, cols]).astype(BFNP),
            "wv": np.ascontiguousarray(W_v[:, cols]).astype(BFNP),
            "wot": np.ascontiguousarray(W_o[:, cols].T),
        })
    return in_maps


def kernel(x, W_q, W_k, W_v, W_o):
    global _cached_nc
    if _cached_nc is None:
        _cached_nc = _build()
    nc = _cached_nc

    in_maps = make_in_maps(x, W_q, W_k, W_v, W_o)
    res = run_bass_kernel_spmd(nc, in_maps, list(range(NCORES))).results
    out = np.empty((B, S, D), np.float32)
    for b in range(B):
        out[b] = res[2 * b]["out"] + res[2 * b + 1]["out"]
    return out
